# revision 2
# baseline (speedup 1.0000x reference)
# Fused single-launch Trainium2 kernel for nn_DeltaNet (B=4, L=4096, D=1024, H=4).
# 8 cores = 4 batches x 2 head-groups. Everything on-device:
#   GEMM (qkv+beta) -> causal convs (PE diag / X4-shift matmuls) -> silu ->
#   chunkwise delta rule (chunk=128, block-doubling inverse on PE) ->
#   local/mid convs -> router stats -> pair AllGather feats -> router MLP ->
#   pair AllReduce logits -> softmax/mix/RMS norms -> pair AllGather o_n ->
#   column-parallel output projection.
import sys, os, json, types
sys.path.insert(0, '/opt/trn_rl_repo')
import numpy as np

B, L, D, H = 4, 4096, 1024, 4
dh = 256
NH = 2
NHD = 512            # channels per core (2 heads)
SEG = 512
NSEG = L // SEG      # 8
C = 128              # delta chunk
CPS = SEG // C       # 4 chunks per segment
HALO = 28            # mid conv needs 24 back-tokens; pad to 28
PREW = 3 + SEG       # pre-activation tile width (halo 3 for K=4 conv)
VW = HALO + SEG + 4  # v post tile width (28 halo + 512 + 4 zero pad)
CWP = 1664           # qkv(1536) + beta(2) padded to 13*128
RKP = 1152           # router contraction padded (1024 + 56 -> 9*128)
RMP = 1152           # router hidden half padded (1080 -> 9*128)
PAIRS = [[0, 1], [2, 3], [4, 5], [6, 7]]

_NC_CACHE = {}
LAST_EXEC_NS = None


def _split_multiwaits(d):
    ctr = [0]
    for f in d['functions']:
        for bb in f['blocks']:
            newlist = []
            for ins in bb['instructions']:
                si = ins.get('sync_info')
                waits = (si or {}).get('on_wait') or []
                if len(waits) > 1:
                    for w in waits[:-1]:
                        ctr[0] += 1
                        newlist.append({
                            "debug": ins.get("debug", 0),
                            "engine": ins["engine"],
                            "ins": [], "outs": [],
                            "name": f"I-mwfix-{ctr[0]}",
                            "opcode": "NoOp",
                            "sync_info": {"on_update": [], "on_wait": [w]},
                        })
                    si['on_wait'] = [waits[-1]]
                newlist.append(ins)
            bb['instructions'] = newlist
    return d


def _patch_nc(nc):
    orig = nc.to_json_bytes
    def patched(self):
        return json.dumps(_split_multiwaits(json.loads(orig()))).encode()
    nc.to_json_bytes = types.MethodType(patched, nc)
    return nc


def _build_nc():
    from contextlib import ExitStack
    import concourse.bass as bass
    import concourse.tile as tile
    import concourse.mybir as mybir

    f32 = mybir.dt.float32
    bf16 = mybir.dt.bfloat16
    AF = mybir.ActivationFunctionType
    ALU = mybir.AluOpType

    nc = bass.Bass(num_devices=8)
    xT = nc.declare_dram_parameter("xT", [D, L], bf16, isOutput=False)
    Wcat = nc.declare_dram_parameter("Wcat", [D, CWP], bf16, isOutput=False)
    Wr1 = nc.declare_dram_parameter("Wr1", [RKP, RMP], bf16, isOutput=False)
    Wr2 = nc.declare_dram_parameter("Wr2", [RMP, 16], bf16, isOutput=False)
    Wo = nc.declare_dram_parameter("Wo", [D, NHD], bf16, isOutput=False)
    DQKV = nc.declare_dram_parameter("DQKV", [128, 48 * 128], bf16, isOutput=False)
    WMID = nc.declare_dram_parameter("WMID", [128, 112 * 32], bf16, isOutput=False)
    WLOC = nc.declare_dram_parameter("WLOC", [128, 32 * 32], bf16, isOutput=False)
    EYEB = nc.declare_dram_parameter("EYEB", [128, 128], bf16, isOutput=False)
    EYEF = nc.declare_dram_parameter("EYEF", [128, 128], f32, isOutput=False)
    BR2 = nc.declare_dram_parameter("BR2", [16, 1], f32, isOutput=False)
    PERM = nc.declare_dram_parameter("PERM", [16, 16], f32, isOutput=False)
    MIXW = nc.declare_dram_parameter("MIXW", [128, NHD], f32, isOutput=False)
    ONW = nc.declare_dram_parameter("ONW", [128, dh], f32, isOutput=False)
    OUT = nc.declare_dram_parameter("out", [L, NHD], f32, isOutput=True)
    DBG = os.environ.get("KERNEL_DBG", "0") == "1"
    if DBG:
        DPATHS = nc.declare_dram_parameter("dpaths", [2 * 128, 1024], f32, isOutput=True)
        DPOST = nc.declare_dram_parameter("dpost", [3 * 128, SEG], f32, isOutput=True)
        DFT = nc.declare_dram_parameter("dft", [2 * 128, 32], f32, isOutput=True)
        DRK8 = nc.declare_dram_parameter("drk8", [128, SEG], f32, isOutput=True)
        DLG = nc.declare_dram_parameter("dlg", [16, SEG], f32, isOutput=True)
        DPT = nc.declare_dram_parameter("dpt", [128, 8], f32, isOutput=True)
        DBT = nc.declare_dram_parameter("dbt", [128, 4], f32, isOutput=True)

    with tile.TileContext(nc) as tc, ExitStack() as ctx:
        # ---------------- pools
        wpool = ctx.enter_context(tc.tile_pool(name="w", bufs=1))
        const = ctx.enter_context(tc.tile_pool(name="const", bufs=1))
        spool = ctx.enter_context(tc.tile_pool(name="state", bufs=1))
        gx = ctx.enter_context(tc.tile_pool(name="gx", bufs=2))       # xT stream tiles
        pre = ctx.enter_context(tc.tile_pool(name="pre", bufs=1))
        tails = ctx.enter_context(tc.tile_pool(name="tails", bufs=2))     # preact CM
        post = ctx.enter_context(tc.tile_pool(name="post", bufs=2))
        pvp = ctx.enter_context(tc.tile_pool(name="pvp", bufs=1))
        lmp = ctx.enter_context(tc.tile_pool(name="lmp", bufs=1))   # postact CM
        x4p = ctx.enter_context(tc.tile_pool(name="x4", bufs=1))      # v X4
        dtp = ctx.enter_context(tc.tile_pool(name="dtp", bufs=2))     # delta TM small
        dcm = ctx.enter_context(tc.tile_pool(name="dcm", bufs=2))     # delta CM small
        scr = ctx.enter_context(tc.tile_pool(name="scr", bufs=1))     # scratch
        stats = ctx.enter_context(tc.tile_pool(name="stats", bufs=5))
        rtp = ctx.enter_context(tc.tile_pool(name="rtp", bufs=1))
        h1p = ctx.enter_context(tc.tile_pool(name="h1p", bufs=3))     # router tiles
        mixp = ctx.enter_context(tc.tile_pool(name="mix", bufs=2))
        onp = ctx.enter_context(tc.tile_pool(name="onp", bufs=1))
        outp = ctx.enter_context(tc.tile_pool(name="outp", bufs=1))
        dram = ctx.enter_context(tc.tile_pool(name="dram", bufs=2, space="DRAM"))
        psA = ctx.enter_context(tc.tile_pool(name="psA", bufs=3, space="PSUM"))
        psB = ctx.enter_context(tc.tile_pool(name="psB", bufs=4, space="PSUM"))
        psT = ctx.enter_context(tc.tile_pool(name="psT", bufs=1, space="PSUM"))

        # ---------------- resident weights
        wr2 = wpool.tile([128, 9 * 16], bf16, tag="wr2")
        for k in range(9):
            nc.sync.dma_start(wr2[:, k * 16:(k + 1) * 16], Wr2[k * 128:(k + 1) * 128, :])
        wosb = wpool.tile([128, 8 * NHD], bf16, tag="wosb")
        for k in range(8):
            nc.sync.dma_start(wosb[:, k * NHD:(k + 1) * NHD], Wo[k * 128:(k + 1) * 128, :])
        wloc = wpool.tile([128, 32 * 32], bf16, tag="wloc")
        nc.sync.dma_start(wloc[:, :], WLOC[:, :])
        eyeb = const.tile([128, 128], bf16, tag="eyeb")
        nc.sync.dma_start(eyeb[:, :], EYEB[:, :])
        eyef = const.tile([128, 128], f32, tag="eyef")
        nc.sync.dma_start(eyef[:, :], EYEF[:, :])
        br2 = const.tile([16, 1], f32, tag="br2")
        nc.sync.dma_start(br2[:, :], BR2[:, :])
        perm = const.tile([16, 16], f32, tag="perm")
        nc.sync.dma_start(perm[:, :], PERM[:, :])
        mixw = const.tile([128, NHD], f32, tag="mixw")
        nc.sync.dma_start(mixw[:, :], MIXW[:, :])
        onw = const.tile([128, dh], f32, tag="onw")
        nc.sync.dma_start(onw[:, :], ONW[:, :])
        epsc = const.tile([128, 2], f32, tag="epsc")
        nc.vector.memset(epsc[:, 0:1], 1e-12)
        nc.vector.memset(epsc[:, 1:2], 1e-5)

        # ---------------- delta state (persistent, per head): S (dk x dv)
        S32 = [spool.tile([128, 2 * dh], f32, tag=f"S32_{h}", name=f"S32_{h}") for h in range(2)]
        Sbf = [spool.tile([128, 2 * dh], bf16, tag=f"Sbf_{h}", name=f"Sbf_{h}") for h in range(2)]
        for h in range(2):
            nc.vector.memset(S32[h][:, :], 0.0)
            nc.vector.memset(Sbf[h][:, :], 0.0)

        ptails = None
        for seg in range(NSEG):
            t0 = seg * SEG
            # ======== GEMM-1: out = Wcat^T @ xT  (CM: cols on partitions)
            xt = gx.tile([128, 8 * SEG], bf16, tag="xt")
            for k in range(8):
                nc.sync.dma_start(xt[:, k * SEG:(k + 1) * SEG],
                                  xT[k * 128:(k + 1) * 128, t0:t0 + SEG])
            newtails = []
            curQ = [pre.tile([128, PREW], bf16, tag=f"preq{i}", name=f"preq{i}") for i in range(4)]
            curK = [pre.tile([128, PREW], bf16, tag=f"prek{i}", name=f"prek{i}") for i in range(4)]
            curV = [pre.tile([128, PREW], bf16, tag=f"prev{i}", name=f"prev{i}") for i in range(4)]
            betas = None
            for m in range(13):
                wcm = gx.tile([128, 8 * 128], bf16, tag="wcm")
                for k in range(8):
                    nc.sync.dma_start(wcm[:, k * 128:(k + 1) * 128],
                                      Wcat[k * 128:(k + 1) * 128, m * 128:(m + 1) * 128])
                ps = psA.tile([128, SEG], f32, tag="psA")
                for k in range(8):
                    nc.tensor.matmul(ps[:, :],
                                     wcm[:, k * 128:(k + 1) * 128],
                                     xt[:, k * SEG:(k + 1) * SEG],
                                     start=(k == 0), stop=(k == 7))
                if m < 12:
                    dst = (curQ + curK + curV)[m]
                    nc.scalar.copy(dst[:, 3:3 + SEG], ps[:, :])
                    if seg > 0:
                        nc.scalar.copy(dst[:, 0:3], ptails[m][:, :])
                    else:
                        nc.vector.memset(dst[:, 0:3], 0.0)
                    ntail = tails.tile([128, 3], bf16, tag=f"ptail{m}")
                    nc.scalar.copy(ntail[:, :], ps[:, SEG - 3:SEG])
                    newtails.append(ntail)
                else:
                    betas = scr.tile([2, SEG], bf16, tag="betas")
                    nc.scalar.activation(betas[:, :], ps[0:2, :], AF.Sigmoid)

            # ======== qkv causal conv (K=4) via diag matmuls + silu
            newvtails = []
            postQ = [post.tile([128, SEG], bf16, tag=f"postq{i}", name=f"postq{i}") for i in range(4)]
            postK = [post.tile([128, SEG], bf16, tag=f"postk{i}", name=f"postk{i}") for i in range(4)]
            postV = [pvp.tile([128, VW], bf16, tag=f"postv{i}", name=f"postv{i}") for i in range(4)]
            for p3, (cur, dstl) in enumerate(((curQ, postQ), (curK, postK), (curV, postV))):
                for i in range(4):
                    dqt = gx.tile([128, 512], bf16, tag="dqt")
                    nc.sync.dma_start(dqt[:, :],
                                      DQKV[:, (p3 * 4 + i) * 512:(p3 * 4 + i + 1) * 512])
                    ps = psA.tile([128, SEG], f32, tag="psA")
                    for tap in range(4):
                        nc.tensor.matmul(ps[:, :],
                                         dqt[:, tap * 128:(tap + 1) * 128],
                                         cur[i][:, tap:tap + SEG],
                                         start=(tap == 0), stop=(tap == 3))
                    if p3 < 2:
                        nc.scalar.activation(dstl[i][:, :], ps[:, :], AF.Silu)
                    else:
                        vt = dstl[i]
                        nc.scalar.activation(vt[:, HALO:HALO + SEG], ps[:, :], AF.Silu)
                        if seg > 0:
                            nc.scalar.copy(vt[:, 0:HALO], vtails[i][:, :])
                        else:
                            nc.vector.memset(vt[:, 0:HALO], 0.0)
                        nc.vector.memset(vt[:, HALO + SEG:VW], 0.0)
                        nvt = tails.tile([128, HALO], bf16, tag=f"vtail{i}")
                        nc.scalar.copy(nvt[:, :], vt[:, SEG:SEG + HALO])
                        newvtails.append(nvt)
            ptails = newtails
            vtails = newvtails
            if DBG and seg == 0:
                dpq = scr.tile([128, SEG], f32, tag="dpq")
                nc.vector.tensor_copy(dpq[:, :], postQ[0][:, :])
                nc.sync.dma_start(DPOST[0:128, :], dpq[:, :])
                nc.vector.tensor_copy(dpq[:, :], postK[0][:, :])
                nc.sync.dma_start(DPOST[128:256, :], dpq[:, :])
                nc.vector.tensor_copy(dpq[:, :], postV[0][:, HALO:HALO + SEG])
                nc.sync.dma_start(DPOST[256:384, :], dpq[:, :])

            # ======== X4 shift-replicated v for local/mid convs
            # X4_g[32*d + ci, u] = vpost[ci of group g, u + d]
            x4 = []
            for g in range(16):
                sub, off = g // 4, (g % 4) * 32
                t = x4p.tile([128, HALO + SEG], bf16, tag=f"x4_{g}", name=f"x4_{g}")
                for d0 in range(4):
                    nc.sync.dma_start(t[32 * d0:32 * d0 + 32, :],
                                      postV[sub][off:off + 32, d0:d0 + HALO + SEG])
                x4.append(t)

            # ======== local/mid convs -> CM sbuf
            locCM = [lmp.tile([128, SEG], bf16, tag=f"loc{i}", name=f"loc{i}") for i in range(4)]
            midCM = [lmp.tile([128, SEG], bf16, tag=f"mid{i}", name=f"mid{i}") for i in range(4)]
            for sub in range(4):
                wmt = gx.tile([128, 28 * 32], bf16, tag="wmt")
                nc.sync.dma_start(wmt[:, :],
                                  WMID[:, sub * 28 * 32:(sub + 1) * 28 * 32])
                psl = psA.tile([128, SEG], f32, tag="psA")
                psm = psA.tile([128, SEG], f32, tag="psA")
                for gg in range(4):
                    g = sub * 4 + gg
                    for j in range(2):  # local: taps 4j+d, rhs offset 22+4j
                        wb = (g * 2 + j)
                        nc.tensor.matmul(psl[32 * gg:32 * gg + 32, :],
                                         wloc[:, wb * 32:(wb + 1) * 32],
                                         x4[g][:, 22 + 4 * j:22 + 4 * j + SEG],
                                         start=(j == 0), stop=(j == 1),
                                         tile_position=(0, 32 * gg))
                    for j in range(7):  # mid: taps 4j+d, rhs offset 4+4j
                        wb = (gg * 7 + j)
                        nc.tensor.matmul(psm[32 * gg:32 * gg + 32, :],
                                         wmt[:, wb * 32:(wb + 1) * 32],
                                         x4[g][:, 4 + 4 * j:4 + 4 * j + SEG],
                                         start=(j == 0), stop=(j == 6),
                                         tile_position=(0, 32 * gg))
                nc.scalar.copy(locCM[sub][:, :], psl[:, :])
                nc.scalar.copy(midCM[sub][:, :], psm[:, :])

            # ======== per-chunk: beta transpose, delta rule, stats, feats
            segfeats = []   # per chunk: ftT tile (32,128) rows = 14*hl + f
            segpaths = []   # per (chunk, head): paths4 TM tile
            for ci in range(CPS):
                cof = ci * C  # chunk offset within segment
                # beta: (2, C) rows -> (C, 2) cols
                pbt = psT.tile([128, 16], bf16, tag="psT")
                nc.tensor.transpose(pbt[:, 0:2], betas[:, cof:cof + C], eyeb[0:2, 0:2])
                betaT = dtp.tile([128, 4], f32, tag="betaT")
                nc.scalar.copy(betaT[:, 0:2], pbt[:, 0:2])
                nc.vector.tensor_scalar_mul(betaT[:, 2:4], betaT[:, 0:2], -1.0)

                ftTs = []
                for h in range(2):
                    qs0, qs1 = postQ[2 * h], postQ[2 * h + 1]
                    ks0, ks1 = postK[2 * h], postK[2 * h + 1]
                    # ---- TM tiles via DMA transpose
                    paths4 = stats.tile([128, 1024], bf16, tag=f"paths4_{h}")
                    qTM = dtp.tile([128, dh], bf16, tag=f"qTM_{h}")
                    kTM = dtp.tile([128, dh], bf16, tag=f"kTM_{h}")
                    nc.sync.dma_start_transpose(qTM[:, 0:128], qs0[:, cof:cof + C])
                    nc.sync.dma_start_transpose(qTM[:, 128:256], qs1[:, cof:cof + C])
                    nc.sync.dma_start_transpose(kTM[:, 0:128], ks0[:, cof:cof + C])
                    nc.sync.dma_start_transpose(kTM[:, 128:256], ks1[:, cof:cof + C])
                    nc.sync.dma_start_transpose(
                        paths4[:, 768:896], postV[2 * h][:, HALO + cof:HALO + cof + C])
                    nc.sync.dma_start_transpose(
                        paths4[:, 896:1024], postV[2 * h + 1][:, HALO + cof:HALO + cof + C])
                    nc.sync.dma_start_transpose(paths4[:, 0:128], locCM[2 * h][:, cof:cof + C])
                    nc.sync.dma_start_transpose(paths4[:, 128:256], locCM[2 * h + 1][:, cof:cof + C])
                    nc.sync.dma_start_transpose(paths4[:, 256:384], midCM[2 * h][:, cof:cof + C])
                    nc.sync.dma_start_transpose(paths4[:, 384:512], midCM[2 * h + 1][:, cof:cof + C])
                    # ---- l2 norms (TM)
                    nsc = scr.tile([128, dh], f32, tag="nsc")
                    nq = dtp.tile([128, 8], f32, tag="nq")
                    nc.scalar.activation(nsc[:, :], qTM[:, :], AF.Square,
                                         accum_out=nq[:, 0:1])
                    nc.scalar.activation(nsc[:, :], kTM[:, :], AF.Square,
                                         accum_out=nq[:, 1:2])
                    nc.scalar.activation(nq[:, 2:4], nq[:, 0:2], AF.Sqrt,
                                         bias=epsc[:, 0:1])
                    nc.vector.reciprocal(nq[:, 4:6], nq[:, 2:4])
                    rnq, rnk = nq[:, 4:5], nq[:, 5:6]
                    # ---- scaled tensors
                    khat = dtp.tile([128, dh], bf16, tag=f"khat_{h}")
                    nc.vector.tensor_scalar_mul(khat[:, :], kTM[:, :], rnk)
                    uw = dtp.tile([128, 2 * dh], bf16, tag=f"uw_{h}")
                    nc.vector.tensor_scalar_mul(uw[:, 0:dh], paths4[:, 768:1024],
                                                betaT[:, h:h + 1])
                    nc.vector.tensor_scalar_mul(uw[:, dh:2 * dh], khat[:, :],
                                                betaT[:, h:h + 1])
                    # ---- k-hat transposed (CM)
                    khatT = dcm.tile([128, dh], bf16, tag=f"khatT_{h}")
                    nc.sync.dma_start_transpose(khatT[:, 0:128], khat[:, 0:128])
                    nc.sync.dma_start_transpose(khatT[:, 128:256], khat[:, 128:256])
                    # ---- M' = khat @ khat^T ; Z = -diag(beta) M'
                    psM = psB.tile([128, C], f32, tag="psB")
                    nc.tensor.matmul(psM[:, :], khatT[:, 0:128], khatT[:, 0:128],
                                     start=True, stop=False)
                    nc.tensor.matmul(psM[:, :], khatT[:, 128:256], khatT[:, 128:256],
                                     start=False, stop=True)
                    Zb = dcm.tile([128, C], bf16, tag=f"Zb_{h}")
                    nc.scalar.activation(Zb[:, :], psM[:, :], AF.Copy,
                                         scale=betaT[:, 2 + h:3 + h])
                    Ab = dcm.tile([128, C], bf16, tag=f"Ab_{h}")
                    nc.gpsimd.affine_select(Ab[:, :], Zb[:, :], [[-1, C]],
                                            ALU.is_gt, 0.0, base=0, channel_multiplier=1)
                    Ztb = dcm.tile([128, C], bf16, tag=f"Ztb_{h}")
                    nc.sync.dma_start_transpose(Ztb[:, :], Zb[:, :])
                    Atb = dcm.tile([128, C], bf16, tag=f"Atb_{h}")
                    nc.gpsimd.affine_select(Atb[:, :], Ztb[:, :], [[1, C]],
                                            ALU.is_gt, 0.0, base=0, channel_multiplier=-1)
                    # ---- doubling: Tt = (I - A)^-T  (A strictly lower)
                    St = dcm.tile([128, C], bf16, tag=f"St_{h}")
                    nc.vector.tensor_add(St[:, :], Atb[:, :], eyeb[:, :])
                    Pc = dcm.tile([128, C], bf16, tag=f"Pc_{h}")
                    Ptc = dcm.tile([128, C], bf16, tag=f"Ptc_{h}")
                    psP = psB.tile([128, C], f32, tag="psB")
                    nc.tensor.matmul(psP[:, :], Atb[:, :], Ab[:, :], start=True, stop=True)
                    nc.scalar.copy(Pc[:, :], psP[:, :])
                    psPt = psB.tile([128, C], f32, tag="psB")
                    nc.tensor.matmul(psPt[:, :], Ab[:, :], Atb[:, :], start=True, stop=True)
                    nc.scalar.copy(Ptc[:, :], psPt[:, :])
                    for j in range(1, 7):
                        psX = psB.tile([128, C], f32, tag="psB")
                        nc.tensor.matmul(psX[:, :], Pc[:, :], St[:, :], start=True, stop=True)
                        St2 = dcm.tile([128, C], bf16, tag=f"St_{h}")
                        nc.vector.tensor_add(St2[:, :], St[:, :], psX[:, :])
                        St = St2
                        if j < 6:
                            psP = psB.tile([128, C], f32, tag="psB")
                            nc.tensor.matmul(psP[:, :], Ptc[:, :], Pc[:, :], start=True, stop=True)
                            psPt = psB.tile([128, C], f32, tag="psB")
                            nc.tensor.matmul(psPt[:, :], Pc[:, :], Ptc[:, :], start=True, stop=True)
                            Pc2 = dcm.tile([128, C], bf16, tag=f"Pc_{h}")
                            nc.scalar.copy(Pc2[:, :], psP[:, :])
                            Ptc2 = dcm.tile([128, C], bf16, tag=f"Ptc_{h}")
                            nc.scalar.copy(Ptc2[:, :], psPt[:, :])
                            Pc, Ptc = Pc2, Ptc2
                    # ---- u,w = Tt^T-contraction; u_i = u - w S
                    psUW = psA.tile([128, 2 * dh], f32, tag="psA")
                    nc.tensor.matmul(psUW[:, :], St[:, :], uw[:, :], start=True, stop=False)
                    psWT = psB.tile([128, 2 * C], f32, tag="psB")
                    nc.tensor.matmul(psWT[:, 0:C], uw[:, dh:dh + 128], St[:, :],
                                     start=True, stop=True)
                    nc.tensor.matmul(psWT[:, C:2 * C], uw[:, dh + 128:2 * dh], St[:, :],
                                     start=True, stop=True)
                    wtn = dcm.tile([128, 2 * C], bf16, tag=f"wtn_{h}")
                    nc.scalar.mul(wtn[:, :], psWT[:, :], -1.0)
                    nc.tensor.matmul(psUW[:, 0:dh], wtn[:, 0:C], Sbf[h][:, 0:dh],
                                     start=False, stop=False)
                    nc.tensor.matmul(psUW[:, 0:dh], wtn[:, C:2 * C], Sbf[h][:, dh:2 * dh],
                                     start=False, stop=True)
                    ui = dtp.tile([128, dh], bf16, tag=f"ui_{h}")
                    nc.scalar.copy(ui[:, :], psUW[:, 0:dh])
                    # ---- attn^T (masked, unnormalized q)
                    psAT = psB.tile([128, C], f32, tag="psB")
                    nc.tensor.matmul(psAT[:, :], khatT[:, 0:128], qs0[:, cof:cof + C],
                                     start=True, stop=False)
                    nc.tensor.matmul(psAT[:, :], khatT[:, 128:256], qs1[:, cof:cof + C],
                                     start=False, stop=True)
                    atr = dcm.tile([128, C], bf16, tag=f"atr_{h}")
                    nc.scalar.copy(atr[:, :], psAT[:, :])
                    atm = dcm.tile([128, C], bf16, tag=f"atm_{h}")
                    nc.gpsimd.affine_select(atm[:, :], atr[:, :], [[1, C]],
                                            ALU.is_ge, 0.0, base=0, channel_multiplier=-1)
                    # ---- o = rnq * (q S + attn^T-contraction u_i)
                    psO = psB.tile([128, dh], f32, tag="psB")
                    nc.tensor.matmul(psO[:, :], qs0[:, cof:cof + C], Sbf[h][:, 0:dh],
                                     start=True, stop=False)
                    nc.tensor.matmul(psO[:, :], qs1[:, cof:cof + C], Sbf[h][:, dh:2 * dh],
                                     start=False, stop=False)
                    nc.tensor.matmul(psO[:, :], atm[:, :], ui[:, :],
                                     start=False, stop=True)
                    nc.vector.tensor_scalar_mul(paths4[:, 512:768], psO[:, :], rnq)
                    # ---- S += k^T u_i
                    psSU = psA.tile([128, 2 * dh], f32, tag="psA")
                    nc.tensor.matmul(psSU[:, 0:dh], khat[:, 0:128], ui[:, :],
                                     start=True, stop=True)
                    nc.tensor.matmul(psSU[:, dh:2 * dh], khat[:, 128:256], ui[:, :],
                                     start=True, stop=True)
                    nc.vector.tensor_add(S32[h][:, :], S32[h][:, :], psSU[:, :])
                    nc.scalar.copy(Sbf[h][:, :], S32[h][:, :])

                    # ---- router stats for this (chunk, head)
                    ft = stats.tile([128, 32], bf16, tag=f"ft_{h}")
                    st = stats.tile([128, 16], f32, tag=f"st_{h}")
                    sq = scr.tile([128, 1024], bf16, tag="sq")
                    nc.vector.tensor_mul(sq[:, :], paths4[:, :], paths4[:, :])
                    nc.vector.tensor_reduce(
                        st[:, 0:4], paths4[:, :].rearrange("p (a b) -> p a b", a=4),
                        mybir.AxisListType.X, ALU.add)
                    nc.vector.tensor_reduce(
                        st[:, 4:8], sq[:, :].rearrange("p (a b) -> p a b", a=4),
                        mybir.AxisListType.X, ALU.add)
                    dp = scr.tile([128, 1536], bf16, tag="dp")
                    pairs = [(0, 1), (0, 2), (0, 3), (1, 2), (1, 3), (2, 3)]
                    for x, (a, b2) in enumerate(pairs):
                        nc.vector.tensor_mul(dp[:, x * dh:(x + 1) * dh],
                                             paths4[:, a * dh:(a + 1) * dh],
                                             paths4[:, b2 * dh:(b2 + 1) * dh])
                    nc.vector.tensor_reduce(
                        st[:, 8:14], dp[:, :].rearrange("p (a b) -> p a b", a=6),
                        mybir.AxisListType.X, ALU.add)
                    # feats: [mean(4) | var(4) | dots(6)] then interleave via DMA
                    nc.vector.memset(ft[:, 14:32], 0.0)
                    nc.vector.tensor_scalar_mul(ft[:, 0:4], st[:, 0:4], 1.0 / dh)
                    sx2 = stats.tile([128, 4], f32, tag=f"sx2_{h}")
                    nc.vector.tensor_mul(sx2[:, :], st[:, 0:4], st[:, 0:4])
                    sxs = stats.tile([128, 4], f32, tag=f"sxs_{h}")
                    nc.vector.tensor_scalar_mul(sxs[:, :], st[:, 4:8], 1.0 / (dh - 1))
                    nc.vector.scalar_tensor_tensor(
                        ft[:, 4:8], sx2[:, :], -1.0 / (dh * (dh - 1)), sxs[:, :],
                        ALU.mult, ALU.add)
                    nc.vector.tensor_scalar_mul(ft[:, 8:14], st[:, 8:14], 1.0 / dh)
                    nc.vector.memset(ft[:, 14:16], 0.0)
                    psF = psT.tile([128, 128], bf16, tag="psT")
                    nc.tensor.transpose(psF[0:32, :], ft[:, :], eyeb[:, :])
                    ftT = stats.tile([32, 128], bf16, tag=f"ftT_{h}")
                    nc.scalar.copy(ftT[:, :], psF[0:32, :])
                    ftTs.append(ftT)
                    segpaths.append(paths4)
                    if DBG and seg == 0 and ci == 0:
                        dp4 = scr.tile([128, 1024], f32, tag="dp4")
                        nc.vector.tensor_copy(dp4[:, :], paths4[:, :])
                        nc.sync.dma_start(DPATHS[h * 128:(h + 1) * 128, :], dp4[:, :])
                        dft_ = scr.tile([128, 32], f32, tag="dft_")
                        nc.vector.tensor_copy(dft_[:, :], ft[:, :])
                        nc.sync.dma_start(DFT[h * 128:(h + 1) * 128, :], dft_[:, :])
                        if h == 0:
                            nc.sync.dma_start(DBT[:, :], betaT[:, :])
                segfeats.append(ftTs)

            # ======== feats AllGather (pair) and rfT k8 tile build
            # bounce layout (28, SEG): row 14*hl + f, per-rank; AG -> (56, SEG)
            fin = dram.tile([28, SEG], bf16, tag="fin")
            fout = dram.tile([56, SEG], bf16, tag="fout")
            fsp = dram.tile([56, SEG], bf16, tag="fsp")
            for ci in range(CPS):
                for h in range(2):
                    # rows of ftT: 0..13 = feats f, copy to bounce rows 14h+f
                    nc.sync.dma_start(fin[14 * h:14 * h + 14, ci * C:(ci + 1) * C],
                                      segfeats[ci][h][0:14, :])
            nc.gpsimd.collective_compute(
                "AllGather", ALU.bypass, replica_groups=PAIRS,
                ins=[fin[:, :]], outs=[fout[:, :]])
            # spread rows: fsp[4f + 2hg' + hl] = fout[28hg' + 14hl + f]
            # (DRAM->DRAM strided copy; rows 0..55 are fully covered)
            nc.sync.dma_start(
                fsp[:, :].rearrange("(f g l) n -> g l f n", g=2, l=2),
                fout[:, :].rearrange("(g l f) n -> g l f n", g=2, l=2))
            rk8 = rtp.tile([128, SEG], bf16, tag="rk8")
            nc.vector.memset(rk8[:, :], 0.0)
            nc.sync.dma_start(rk8[0:56, :], fsp[:, :])
            # ======== router GEMM: h1 = silu(rf @ W1half) ; logits partial
            psL = psT.tile([128, SEG], f32, tag="psT")
            for m in range(9):
                wr1t = gx.tile([128, 9 * 128], bf16, tag="wr1t")
                for k in range(9):
                    nc.sync.dma_start(wr1t[:, k * 128:(k + 1) * 128],
                                      Wr1[k * 128:(k + 1) * 128, m * 128:(m + 1) * 128])
                psH = psA.tile([128, SEG], f32, tag="psA")
                for k in range(8):
                    nc.tensor.matmul(psH[:, :],
                                     wr1t[:, k * 128:(k + 1) * 128],
                                     xt[:, k * SEG:(k + 1) * SEG],
                                     start=(k == 0), stop=False)
                nc.tensor.matmul(psH[:, :],
                                 wr1t[:, 8 * 128:9 * 128],
                                 rk8[:, :], start=False, stop=True)
                h1 = h1p.tile([128, SEG], bf16, tag="h1")
                nc.scalar.activation(h1[:, :], psH[:, :], AF.Silu)
                nc.tensor.matmul(psL[0:16, :], wr2[:, m * 16:(m + 1) * 16],
                                 h1[:, :], start=(m == 0), stop=(m == 8))
            lgc = rtp.tile([16, SEG], f32, tag="lgc")
            nc.scalar.copy(lgc[:, :], psL[0:16, :])
            lin = dram.tile([16, SEG], f32, tag="lin")
            lout = dram.tile([16, SEG], f32, tag="lout")
            nc.sync.dma_start(lin[:, :], lgc[:, :])
            nc.gpsimd.collective_compute(
                "AllReduce", ALU.add, replica_groups=PAIRS,
                ins=[lin[:, :]], outs=[lout[:, :]])
            lgr = rtp.tile([16, SEG], f32, tag="lgr")
            nc.sync.dma_start(lgr[:, :], lout[:, :])
            # permute my head-group's 8 rows to 0..7 (per-core PERM input),
            # then add r_b2 (per-core permuted)
            psPm = psB.tile([16, SEG], f32, tag="psB")
            nc.tensor.matmul(psPm[:, :], perm[:, :], lgr[:, :], start=True, stop=True)
            lgp = rtp.tile([16, SEG], f32, tag="lgp")
            nc.scalar.activation(lgp[:, :], psPm[:, :], AF.Identity,
                                 bias=br2[:, 0:1])
            if DBG and seg == 0:
                drk = scr.tile([128, SEG], f32, tag="drk")
                nc.vector.tensor_copy(drk[:, :], rk8[:, :])
                nc.sync.dma_start(DRK8[:, :], drk[:, :])
                nc.sync.dma_start(DLG[:, :], lgp[:, :])

            # ======== softmax + floor + mix + norms + o_n (per chunk)
            onCM = [onp.tile([128, SEG], bf16, tag=f"onCM{i}", name=f"onCM{i}") for i in range(4)]
            for ci in range(CPS):
                cof = ci * C
                pbt = psT.tile([128, 16], f32, tag="psT")
                nc.tensor.transpose(pbt[:, 0:8], lgp[0:8, cof:cof + C], eyef[0:8, 0:8])
                et = mixp.tile([128, 8], f32, tag="et")
                nc.scalar.activation(et[:, :], pbt[:, 0:8], AF.Exp)
                sm = mixp.tile([128, 4], f32, tag="sm")
                nc.vector.tensor_reduce(
                    sm[:, 0:2], et[:, :].rearrange("p (a b) -> p a b", a=2),
                    mybir.AxisListType.X, ALU.add)
                nc.vector.reciprocal(sm[:, 2:4], sm[:, 0:2])
                pt = mixp.tile([128, 8], f32, tag="pt")
                for hh in range(2):
                    nc.vector.tensor_scalar_mul(pt[:, 4 * hh:4 * hh + 4],
                                                et[:, 4 * hh:4 * hh + 4],
                                                sm[:, 2 + hh:3 + hh])
                nc.scalar.activation(pt[:, :], pt[:, :], AF.Copy,
                                     bias=0.01, scale=0.96)
                if DBG and seg == 0 and ci == 0:
                    nc.sync.dma_start(DPT[:, :], pt[:, :])
                for h in range(2):
                    paths4 = segpaths[ci * 2 + h]
                    mx = mixp.tile([128, dh], f32, tag=f"mx_{h}")
                    nc.vector.tensor_scalar_mul(mx[:, :], paths4[:, 0:dh],
                                                pt[:, 4 * h + 0:4 * h + 1])
                    nc.vector.scalar_tensor_tensor(
                        mx[:, :], paths4[:, dh:2 * dh], pt[:, 4 * h + 1:4 * h + 2],
                        mx[:, :], ALU.mult, ALU.add)
                    nc.vector.scalar_tensor_tensor(
                        mx[:, :], paths4[:, 2 * dh:3 * dh], pt[:, 4 * h + 2:4 * h + 3],
                        mx[:, :], ALU.mult, ALU.add)
                    nc.vector.scalar_tensor_tensor(
                        mx[:, :], paths4[:, 3 * dh:4 * dh], pt[:, 4 * h + 3:4 * h + 4],
                        mx[:, :], ALU.mult, ALU.add)
                    # MixNorm + onorm
                    nrm = mixp.tile([128, 8], f32, tag=f"nrm_{h}")
                    nsc2 = scr.tile([128, dh], f32, tag="nsc2")
                    nc.scalar.activation(nsc2[:, :], mx[:, :], AF.Square,
                                         accum_out=nrm[:, 0:1])
                    nc.scalar.activation(nrm[:, 1:2], nrm[:, 0:1], AF.Sqrt,
                                         scale=1.0 / dh, bias=epsc[:, 1:2])
                    nc.vector.reciprocal(nrm[:, 2:3], nrm[:, 1:2])
                    mx2 = mx
                    nc.vector.scalar_tensor_tensor(
                        mx2[:, :], mx[:, :], nrm[:, 2:3], mixw[:, h * dh:(h + 1) * dh],
                        ALU.mult, ALU.mult)
                    nc.scalar.activation(nsc2[:, :], mx2[:, :], AF.Square,
                                         accum_out=nrm[:, 4:5])
                    nc.scalar.activation(nrm[:, 5:6], nrm[:, 4:5], AF.Sqrt,
                                         scale=1.0 / dh, bias=epsc[:, 1:2])
                    nc.vector.reciprocal(nrm[:, 6:7], nrm[:, 5:6])
                    onTM = mixp.tile([128, dh], bf16, tag=f"onTM_{h}")
                    nc.vector.scalar_tensor_tensor(
                        onTM[:, :], mx2[:, :], nrm[:, 6:7], onw[:, :],
                        ALU.mult, ALU.mult)
                    # o_n -> CM
                    nc.sync.dma_start_transpose(
                        onCM[2 * h][:, cof:cof + C], onTM[:, 0:128])
                    nc.sync.dma_start_transpose(
                        onCM[2 * h + 1][:, cof:cof + C], onTM[:, 128:256])

            # ======== o_n AllGather + output projection
            oin = dram.tile([NHD, SEG], bf16, tag="oin")
            oout = dram.tile([2 * NHD, SEG], bf16, tag="oout")
            for i in range(4):
                nc.sync.dma_start(oin[i * 128:(i + 1) * 128, :], onCM[i][:, :])
            nc.gpsimd.collective_compute(
                "AllGather", ALU.bypass, replica_groups=PAIRS,
                ins=[oin[:, :]], outs=[oout[:, :]])
            onf = [outp.tile([128, SEG], bf16, tag=f"onf{i}", name=f"onf{i}") for i in range(8)]
            for i in range(8):
                nc.sync.dma_start(onf[i][:, :], oout[i * 128:(i + 1) * 128, :])
            for ci in range(CPS):
                cof = ci * C
                psO2 = psA.tile([128, NHD], f32, tag="psA")
                for k in range(8):
                    nc.tensor.matmul(psO2[:, :], onf[k][:, cof:cof + C],
                                     wosb[:, k * NHD:(k + 1) * NHD],
                                     start=(k == 0), stop=(k == 7))
                osb = outp.tile([128, NHD], f32, tag="osb")
                nc.scalar.copy(osb[:, :], psO2[:, :])
                nc.sync.dma_start(OUT[t0 + cof:t0 + cof + C, :], osb[:, :])

    _patch_nc(nc)
    return nc


# ---------------------------------------------------------------- host side
def _prep_inputs(hidden_states, Wq, Wk, Wv, Wb, conv_q_w, conv_k_w, conv_v_w,
                 local_w, mid_w, r_w1, r_b1, r_w2, r_b2, mix_w, onorm_w, Wo):
    """Build the 8 per-core input maps."""
    f32, bf16 = np.float32, np.dtype('bfloat16') if hasattr(np, 'bfloat16') else None
    import ml_dtypes
    bf16 = ml_dtypes.bfloat16
    hs = np.asarray(hidden_states, f32)
    eye_b = np.eye(128, dtype=f32).astype(bf16)
    eye_f = np.eye(128, dtype=f32)
    in_maps = []
    for core in range(8):
        b, hg = core // 2, core % 2
        cols = slice(hg * NHD, (hg + 1) * NHD)
        # fused qkv + beta weights (CM GEMM streams xT, weights stationary)
        Wcat = np.zeros((D, CWP), f32)
        Wcat[:, 0:512] = Wq[:, cols]
        Wcat[:, 512:1024] = Wk[:, cols]
        Wcat[:, 1024:1536] = Wv[:, cols]
        Wcat[:, 1536:1538] = Wb[:, 2 * hg:2 * hg + 2]
        # router weights: rows [x (1024) | feats (56) | pad], cols = my half.
        # Device feats order per head: [mean4, var4, dots6]; reference is
        # [m_l, v_l, m_m, v_m, m_d, v_d, m_v, v_v, dots6] -> permute rows.
        Wr1 = np.zeros((RKP, RMP), f32)
        Wr1[0:1024, 0:1080] = r_w1[0:1024, hg * 1080:(hg + 1) * 1080]
        fmap = [0, 2, 4, 6, 1, 3, 5, 7, 8, 9, 10, 11, 12, 13]  # f_my -> f_ref
        for fmy in range(14):
            fr = fmap[fmy]
            for hh in range(4):
                Wr1[1024 + 4 * fmy + hh, 0:1080] = \
                    r_w1[1024 + 4 * fr + hh, hg * 1080:(hg + 1) * 1080]
        Wr2 = np.zeros((RMP, 16), f32)
        Wr2[0:1080, :] = r_w2[hg * 1080:(hg + 1) * 1080, :]
        # qkv conv diag matrices: (path, mtile, tap) -> diag(w[128ch, tap])
        DQ = np.zeros((128, 48 * 128), f32)
        for p3, cw in enumerate((conv_q_w, conv_k_w, conv_v_w)):
            wloc_ = np.asarray(cw, f32)[cols]  # (512, 4)
            for i in range(4):
                for tap in range(4):
                    di = (p3 * 4 + i) * 4 + tap
                    DQ[:, di * 128:(di + 1) * 128] = np.diag(wloc_[i * 128:(i + 1) * 128, tap])
        # mid conv weight blocks: (g, j): lhsT[32d+ci, co] = w[32g+co, 4j+d]*(ci==co)
        WM = np.zeros((128, 112 * 32), f32)
        wm = np.asarray(mid_w, f32)[cols]  # (512, 25)
        for g in range(16):
            for j in range(7):
                blk = np.zeros((128, 32), f32)
                for d0 in range(4):
                    tap = 4 * j + d0
                    if tap < 25:
                        for ci in range(32):
                            blk[32 * d0 + ci, ci] = wm[32 * g + ci, tap]
                WM[:, (g * 7 + j) * 32:(g * 7 + j + 1) * 32] = blk
        WL = np.zeros((128, 32 * 32), f32)
        wl = np.asarray(local_w, f32)[cols]  # (512, 7)
        for g in range(16):
            for j in range(2):
                blk = np.zeros((128, 32), f32)
                for d0 in range(4):
                    tap = 4 * j + d0
                    if tap < 7:
                        for ci in range(32):
                            blk[32 * d0 + ci, ci] = wl[32 * g + ci, tap]
                WL[:, (g * 2 + j) * 32:(g * 2 + j + 1) * 32] = blk
        # head-group permutation + permuted bias
        P = np.zeros((16, 16), f32)
        for m in range(8):
            P[8 * hg + m, m] = 1.0  # lhsT[r, m] = 1 iff r = 8*hg + m
        br2p = np.zeros((16, 1), f32)
        br2p[0:8, 0] = np.asarray(r_b2, f32).reshape(-1)[8 * hg:8 * hg + 8]
        mixw_bc = np.zeros((128, NHD), f32)
        for hl in range(2):
            mixw_bc[:, hl * dh:(hl + 1) * dh] = np.asarray(mix_w, f32)[2 * hg + hl][None, :]
        onw_bc = np.broadcast_to(np.asarray(onorm_w, f32)[None, :], (128, dh)).copy()
        in_maps.append({
            "xT": np.ascontiguousarray(hs[b].T).astype(bf16),
            "Wcat": Wcat.astype(bf16),
            "Wr1": Wr1.astype(bf16),
            "Wr2": Wr2.astype(bf16),
            "Wo": np.ascontiguousarray(np.asarray(Wo, f32)[:, cols]).astype(bf16),
            "DQKV": DQ.astype(np.float16),
            "WMID": WM.astype(np.float16),
            "WLOC": WL.astype(np.float16),
            "EYEB": eye_b,
            "EYEF": eye_f,
            "BR2": br2p,
            "PERM": P,
            "MIXW": mixw_bc,
            "ONW": onw_bc,
        })
    return in_maps


def kernel(hidden_states, Wq, Wk, Wv, Wb, conv_q_w, conv_k_w, conv_v_w,
           local_w, mid_w, r_w1, r_b1, r_w2, r_b2, mix_w, onorm_w, Wo):
    global LAST_EXEC_NS
    from concourse.bass_utils import run_bass_kernel_spmd
    if 'fused' not in _NC_CACHE:
        _NC_CACHE['fused'] = _build_nc()
    nc = _NC_CACHE['fused']
    in_maps = _prep_inputs(hidden_states, Wq, Wk, Wv, Wb, conv_q_w, conv_k_w,
                           conv_v_w, local_w, mid_w, r_w1, r_b1, r_w2, r_b2,
                           mix_w, onorm_w, Wo)
    import time as _time
    trace = os.environ.get("KERNEL_TRACE", "0") == "1"
    t0 = _time.time()
    try:
        res = run_bass_kernel_spmd(nc, in_maps, core_ids=list(range(8)), trace=trace)
    except ModuleNotFoundError:
        # NTFF profiling hook unavailable in this container; run untraced.
        res = run_bass_kernel_spmd(nc, in_maps, core_ids=list(range(8)), trace=False)
    t1 = _time.time()
    if getattr(res, 'exec_time_ns', None):
        LAST_EXEC_NS = res.exec_time_ns
    else:
        LAST_EXEC_NS = int((t1 - t0) * 1e9)  # wall clock of device dispatch
    out = np.zeros((B, L, D), np.float32)
    for core in range(8):
        b, hg = core // 2, core % 2
        out[b][:, hg * NHD:(hg + 1) * NHD] = res.results[core]["out"]
    return out
